# revision 18
# baseline (speedup 1.0000x reference)
"""Trainium2 Bass kernel for the BiologicalRNN problem.

Reference semantics (see problem reference):
    u_seq = einsum('btd,hd->tbh', x, W_hi_eff)        # input drive
    h_{t+1} = h_t + alpha*(-h_t + u_t + elu(h_t) @ W_hh^T + b)
    outs[t] = h_{t+1} @ W_oh_eff^T
    returns (outs.transpose(1,0,2), h_last)

Strategy: data-parallel over batch B=256 across 8 NeuronCores (32 each).
Per core everything lives in a transposed "P1" layout: a [128, 256] tile
where element [p, 32*k + b] = v[b, 128*k + p] for a [32, 1024] matrix.
The recurrent matmul runs weights-stationary: for each output chunk m,
psum[m] [128,32] accumulates over 8 contraction chunks k with
lhsT = (alpha*W_hh)^T tile (k,m) in bf16 and rhs = elu(h)^T chunk k.
The elementwise update+elu runs on DVE/ACT per 128-column chunk, fully
overlapped with the tensor engine. The -1 in elu (expm1) is folded into
the precomputed drive: u''_t = alpha*(u_t + b) - alpha*W_hh.sum(1).
"""

import math
import numpy as np

import concourse.bass as bass
import concourse.mybir as mybir
import concourse.tile as tile
from concourse.bass import ds, ts
from concourse.bass_utils import run_bass_kernel_spmd

AF = mybir.ActivationFunctionType
ALU = mybir.AluOpType
F32 = mybir.dt.float32
BF16 = mybir.dt.bfloat16

def _patch_tile_drain(max_drain_waits=1):
    """walrus CoreV3 codegen rejects the Tile end-of-kernel Drain when it
    carries several sem waits ("Too many sync wait commands"). Keep one wait
    on the drain and re-emit the rest as standalone SP wait_ge instructions;
    the all-engine barrier right after preserves semantics."""
    from concourse.vector_clock import ScopedClock

    if getattr(tile.TileContext, "_drain_patched", False):
        return

    def _drain_and_barrier(self, tick_clock, wait_clock):
        nc = self.nc
        drain_inst = nc.sync.drain()
        wait_clock.add_sem_waits(
            drain_inst.ins, ScopedClock({None: tick_clock.global_clock})
        )
        si = drain_inst.ins.sync_info
        waits = list(si.on_wait) if si is not None else []
        if len(waits) > max_drain_waits:
            si.on_wait = waits[:max_drain_waits]
            handles = {h.name: h for h in self.sems.allocated().values()}
            for w in waits[max_drain_waits:]:
                nc.sync.wait_ge(handles[w.ant_name], w.wait_value)

        nc.all_engine_barrier()
        assert self.sems is not None
        popped = nc._tile_sem_poison_stack.pop()
        assert popped is self._sem_poison
        nc.clear_and_free_semaphores(list(self.sems.allocated().values()))
        nc.all_engine_barrier()

    tile.TileContext._drain_and_barrier = _drain_and_barrier
    tile.TileContext._drain_patched = True


_patch_tile_drain()

WAIT_LIMIT = 1


def _split_excess_waits(nc, limit=WAIT_LIMIT):
    """walrus CoreV3 codegen accepts only a couple of sem waits per
    instruction. Hoist the excess onto NoOp instructions inserted right
    before the offending instruction on the same engine."""
    def _nwaits(i):
        try:
            si = i.sync_info
        except Exception:
            return 0
        return len(si.on_wait) if si is not None else 0

    for f in nc.m.functions:
        for bb in f.blocks:
            il = bb.instructions
            if not any(_nwaits(i) > limit for i in il):
                continue
            new = []
            for inst in il:
                if _nwaits(inst) == 0:
                    new.append(inst)
                    continue
                si = inst.sync_info
                waits = list(si.on_wait)
                if len(waits) > limit:
                    extra = waits[:-limit]
                    si.on_wait = waits[-limit:]
                    for i0 in range(0, len(extra), limit):
                        nop = mybir.InstNoOp(
                            name=f"{inst.name}-wsplit{i0}",
                            engine=inst.engine,
                            bass_nofuse=True,
                            sync_info=mybir.SyncInfo(
                                on_wait=extra[i0:i0 + limit], on_update=[]
                            ),
                        )
                        new.append(nop)
                new.append(inst)
            il[:] = new


N_CORES = 8
B_FULL = 256
T_FULL = 512
H = 1024
D_IN = 66
O = 8
DT = 0.02
TAU = 0.1
ALPHA = DT / TAU          # 0.2
BL = B_FULL // N_CORES    # 32 batch per core
HK = H // 128             # 8 hidden chunks
TB = 16                   # timesteps per phase-1/3 block (16*32 = 512 cols)
U = 8                     # recurrent steps unrolled per hw-loop iteration


def build_program(T=T_FULL):
    """Builds the per-core Bass program (SPMD: same program on all cores)."""
    assert T % U == 0 and T % TB == 0
    nc = bass.Bass("TRN2", target_bir_lowering=False, debug=False,
                   num_devices=N_CORES)

    # ---- I/O ----
    xt = nc.dram_tensor("xt", [T, D_IN, BL], F32, kind="ExternalInput")
    whi = nc.dram_tensor("whi", [HK, D_IN, 128], F32, kind="ExternalInput")
    whh16 = nc.dram_tensor("whh16", [HK * HK, 128, 128], BF16,
                           kind="ExternalInput")
    woh16 = nc.dram_tensor("woh16", [HK, 128, O], BF16, kind="ExternalInput")
    cvec = nc.dram_tensor("cvec", [128, HK], F32, kind="ExternalInput")
    h0 = nc.dram_tensor("h0", [128, HK * BL], F32, kind="ExternalInput")
    eluh0 = nc.dram_tensor("eluh0", [128, HK * BL], BF16, kind="ExternalInput")

    outs_t = nc.dram_tensor("outs_t", [O, T * BL], F32, kind="ExternalOutput")
    hlast = nc.dram_tensor("hlast", [128, HK * BL], F32, kind="ExternalOutput")

    with tile.TileContext(nc) as tc:
        with (
            tc.tile_pool(name="dram", bufs=1, space="DRAM") as dram,
            tc.tile_pool(name="const", bufs=1) as const,
            tc.tile_pool(name="state", bufs=1) as state,
            tc.tile_pool(name="ut", bufs=4) as ut_pool,
            tc.tile_pool(name="tmp", bufs=8) as tmp_pool,
            tc.tile_pool(name="h16", bufs=4) as h16_pool,
        ):
            u_dram = dram.tile([T, 128, HK * BL], F32)
            hh2 = dram.tile([HK, T, 128, BL], BF16)  # h history, k-major

            # ---- constants into SBUF ----
            whi_sb = const.tile([D_IN, HK * 128], F32)
            nc.sync.dma_start(
                out=whi_sb[:].rearrange("d (m f) -> d m f", m=HK),
                in_=whi.rearrange("m d f -> d m f"),
            )
            w16_sb = const.tile([128, HK * HK * 128], BF16)
            nc.sync.dma_start(
                out=w16_sb[:].rearrange("p (n f) -> p n f", n=HK * HK),
                in_=whh16.rearrange("n p f -> p n f"),
            )
            woh_sb = const.tile([128, HK * O], BF16)
            nc.sync.dma_start(
                out=woh_sb[:].rearrange("p (k o) -> p k o", k=HK),
                in_=woh16.rearrange("k p o -> p k o"),
            )
            cv_sb = const.tile([128, HK], F32)
            nc.sync.dma_start(out=cv_sb[:], in_=cvec[:])

            h_a = state.tile([128, HK * BL], F32)
            h_b = state.tile([128, HK * BL], F32)
            eluh_a = state.tile([128, HK * BL], BF16)
            eluh_b = state.tile([128, HK * BL], BF16)
            nc.sync.dma_start(out=h_a[:], in_=h0[:])
            nc.sync.dma_start(out=eluh_a[:], in_=eluh0[:])

            # ---- phase 1: u''[t] = alpha*x_t@W_hi_eff^T + cvec ----
            with (
                tc.tile_pool(name="xts", bufs=3) as xts_pool,
                tc.tile_pool(name="usb", bufs=4) as usb_pool,
                tc.tile_pool(name="psum_u", bufs=2, space="PSUM")
                as psum_u_pool,
            ):
                for tb in range(T // TB):
                    xts = xts_pool.tile([D_IN, TB * BL], F32)
                    nc.sync.dma_start(
                        out=xts[:].rearrange("d (t b) -> d t b", t=TB),
                        in_=xt[ts(tb, TB)].rearrange("t d b -> d t b"),
                    )
                    for m in range(HK):
                        pu = psum_u_pool.tile([128, TB * BL], F32)
                        nc.tensor.matmul(
                            pu[:], whi_sb[:, ts(m, 128)], xts[:],
                            start=True, stop=True,
                        )
                        usb = usb_pool.tile([128, TB * BL], F32)
                        nc.scalar.activation(
                            usb[:], pu[:], AF.Identity,
                            bias=cv_sb[:, ds(m, 1)], scale=1.0,
                        )
                        nc.sync.dma_start(
                            out=u_dram[ts(tb, TB), :, ts(m, BL)]
                            .rearrange("t p b -> p t b"),
                            in_=usb[:].rearrange("p (t b) -> p t b", t=TB),
                        )

            # ---- phase 2: the recurrent scan ----
            def step(tj, h_in, eluh_in, h_out, eluh_out, psum_pool):
                ut = ut_pool.tile([128, HK * BL], F32)
                nc.sync.dma_start(
                    out=ut[:],
                    in_=u_dram[ds(tj, 1)].rearrange("o p b -> p o b"),
                )
                # one PSUM bank per output chunk m so the 8 accumulation
                # groups can interleave (start=True clears a whole bank)
                pss = [psum_pool.tile([128, BL], F32, tag="ps", name="ps")
                       for _ in range(HK)]
                # half A (k=0..3) for all m, then half B (k=4..7): gives the
                # next step ~half a step of runnable matmuls while this
                # step's late elu chunks are still being computed
                for half in range(2):
                    for m in range(HK):
                        for k in range(half * 4, half * 4 + 4):
                            nc.tensor.matmul(
                                pss[m][:],
                                w16_sb[:, ts(m * HK + k, 128)],
                                eluh_in[:, ts(k, BL)],
                                start=(k == 0), stop=(k == HK - 1),
                            )
                h16 = h16_pool.tile([128, HK * BL], BF16)
                # update + elu chains at [128, 64] grain (pairs of m)
                for P in range(4):
                    sp = tmp_pool.tile([128, 2 * BL], F32, tag="s")
                    for i in range(2):
                        m = 2 * P + i
                        nc.vector.tensor_tensor(
                            sp[:, ts(i, BL)], pss[m][:], ut[:, ts(m, BL)],
                            ALU.add,
                        )
                    cp = ds(P * 2 * BL, 2 * BL)
                    # h' = (1-a)*h + (u'' + psum)
                    nc.vector.scalar_tensor_tensor(
                        h_out[:, cp], h_in[:, cp], float(1.0 - ALPHA), sp[:],
                        ALU.mult, ALU.add,
                    )
                    mn = tmp_pool.tile([128, 2 * BL], F32, tag="mn")
                    nc.vector.tensor_scalar_min(mn[:], h_out[:, cp], 0.0)
                    r = tmp_pool.tile([128, 2 * BL], F32, tag="r")
                    nc.vector.tensor_scalar_max(r[:], h_out[:, cp], 0.0)
                    e = tmp_pool.tile([128, 2 * BL], F32, tag="e")
                    nc.scalar.activation(e[:], mn[:], AF.Exp)
                    # elu(h') = (exp(min(h',0)) - 1) + max(h',0), cast bf16
                    nc.vector.scalar_tensor_tensor(
                        eluh_out[:, cp], e[:], -1.0, r[:],
                        ALU.add, ALU.add,
                    )
                    if P % 2 == 1:
                        c = P // 2
                        cs = ds(c * 128, 128)
                        nc.scalar.activation(h16[:, cs], h_out[:, cs],
                                             AF.Copy, bias=0.0, scale=1.0)
                nc.sync.dma_start(
                    out=hh2[:, ds(tj, 1)].rearrange("k o p b -> p k o b"),
                    in_=h16[:].rearrange("p (k b) -> p k () b", k=HK),
                )

            with tc.tile_pool(name="psum", bufs=8, space="PSUM") as psum_pool:
                with tc.For_i(0, T, U,
                              hint_engines=(mybir.EngineType.PE,)) as iv:
                    for j in range(U):
                        if j % 2 == 0:
                            step(iv + j, h_a, eluh_a, h_b, eluh_b, psum_pool)
                        else:
                            step(iv + j, h_b, eluh_b, h_a, eluh_a, psum_pool)

            # after T steps (T even) the live state is back in h_a
            nc.sync.dma_start(out=hlast[:], in_=h_a[:])

            # ---- phase 3: outs_t = W_oh_eff @ h_hist^T ----
            with (
                tc.tile_pool(name="hh", bufs=4) as hh_pool,
                tc.tile_pool(name="ob", bufs=4) as ob_pool,
                tc.tile_pool(name="psum_o", bufs=2, space="PSUM")
                as psum_o_pool,
            ):
                for nb in range(T // TB):
                    po = psum_o_pool.tile([O, TB * BL], F32)
                    for k in range(HK):
                        hhs = hh_pool.tile([128, TB * BL], BF16)
                        nc.sync.dma_start(
                            out=hhs[:].rearrange("p (t b) -> p t b", t=TB),
                            in_=hh2[k, ts(nb, TB)].rearrange("t p b -> p t b"),
                        )
                        nc.tensor.matmul(
                            po[:], woh_sb[:, ts(k, O)], hhs[:],
                            start=(k == 0), stop=(k == HK - 1),
                        )
                    ob = ob_pool.tile([O, TB * BL], F32)
                    nc.vector.tensor_copy(ob[:], po[:])
                    nc.sync.dma_start(out=outs_t[:, ts(nb, TB * BL)],
                                      in_=ob[:])

    _split_excess_waits(nc)
    return nc


def _prep_core_inputs(x_shard, W_hi, W_hh, b, W_oh, hidden_init):
    """Host-side packing of one core's inputs (cheap O(H^2) work)."""
    T = x_shard.shape[1]
    half = H // 2
    row = np.arange(H)
    W_hi_eff = np.where(row[:, None] < half, W_hi, 0.0).astype(np.float32)
    W_oh_eff = np.where(row[None, :] >= half, W_oh, 0.0).astype(np.float32)

    xt = np.ascontiguousarray(x_shard.transpose(1, 2, 0))  # [T, D, BL]

    aWhi = (ALPHA * W_hi_eff).astype(np.float32)
    whi = np.ascontiguousarray(
        aWhi.reshape(HK, 128, D_IN).transpose(0, 2, 1)
    )  # [HK, D, 128] : whi[m] = (alpha*W_hi_eff)[128m:128m+128, :].T

    import ml_dtypes
    aWt = (ALPHA * W_hh).astype(np.float32).T  # [h_in, h_out]
    whh16 = np.zeros((HK * HK, 128, 128), np.float32)
    for m in range(HK):
        for k in range(HK):
            whh16[m * HK + k] = aWt[k * 128:(k + 1) * 128,
                                    m * 128:(m + 1) * 128]
    whh16 = whh16.astype(ml_dtypes.bfloat16)

    woh16 = np.ascontiguousarray(
        W_oh_eff.T.reshape(HK, 128, O)
    ).astype(ml_dtypes.bfloat16)  # [HK, 128, O]

    # cvec[p, m] = (alpha*b)[128m + p]
    cv = (ALPHA * b).astype(np.float32)
    cvec = np.ascontiguousarray(cv.reshape(HK, 128).T)

    # P1 packing of the initial state: [p, 32k + b] = h[b, 128k + p]
    h0_full = np.broadcast_to(hidden_init, (BL, H)).astype(np.float32)
    h0 = np.ascontiguousarray(
        h0_full.reshape(BL, HK, 128).transpose(2, 1, 0).reshape(128, HK * BL)
    )
    feed0 = np.maximum(h0, 0.0) + np.expm1(np.minimum(h0, 0.0))
    eluh0 = feed0.astype(ml_dtypes.bfloat16)

    return {
        "xt": xt, "whi": whi, "whh16": whh16, "woh16": woh16,
        "cvec": cvec, "h0": h0, "eluh0": eluh0,
    }


_PROGRAM_CACHE = {}


def run(x, W_hi, W_hh, b, W_oh, hidden_init, T=None, trace=False):
    x = np.asarray(x, np.float32)
    B, T_in, _ = x.shape
    if T is None:
        T = T_in
    assert B == B_FULL
    if T not in _PROGRAM_CACHE:
        _PROGRAM_CACHE[T] = build_program(T)
    nc = _PROGRAM_CACHE[T]

    in_maps = []
    for c in range(N_CORES):
        shard = x[c * BL:(c + 1) * BL, :T]
        in_maps.append(_prep_core_inputs(
            shard, np.asarray(W_hi, np.float32), np.asarray(W_hh, np.float32),
            np.asarray(b, np.float32), np.asarray(W_oh, np.float32),
            np.asarray(hidden_init, np.float32),
        ))

    res = run_bass_kernel_spmd(nc, in_maps, core_ids=list(range(N_CORES)),
                               trace=trace)

    outs = np.empty((B_FULL, T, O), np.float32)
    h_last = np.empty((B_FULL, H), np.float32)
    for c in range(N_CORES):
        ot = res.results[c]["outs_t"]          # [O, T*BL]
        outs[c * BL:(c + 1) * BL] = (
            ot.reshape(O, T, BL).transpose(2, 1, 0)
        )
        hl = res.results[c]["hlast"]           # [128, HK*BL]
        h_last[c * BL:(c + 1) * BL] = (
            hl.reshape(128, HK, BL).transpose(2, 1, 0).reshape(BL, H)
        )
    return (outs, h_last), res


def kernel(x, W_hi, W_hh, b, W_oh, hidden_init):
    (outs, h_last), _ = run(x, W_hi, W_hh, b, W_oh, hidden_init)
    return outs, h_last


# revision 20
# speedup vs baseline: 1.6798x; 1.6798x over previous
"""Trainium2 Bass kernel for the BiologicalRNN problem.

Reference semantics:
    u_seq = einsum('btd,hd->tbh', x, W_hi_eff)        # input drive
    h_{t+1} = h_t + alpha*(-h_t + u_t + elu(h_t) @ W_hh^T + b)
    outs[t] = h_{t+1} @ W_oh_eff^T
    returns (outs.transpose(1,0,2), h_last)

Strategy: data-parallel over batch B=256 across 8 NeuronCores (32 each).
Per core everything lives in a transposed "P1" layout: a [128, 256] tile
where element [p, 32*k + b] = v[b, 128*k + p] for a [32, 1024] matrix.

The recurrent matmul runs weights-stationary in bf16: for each output
chunk m, psum[m] [128,32] accumulates 8 contraction chunks k with
lhsT = (alpha*W_hh)^T tile (k,m) and rhs = elu(h)^T chunk k. The drive
u''_t = alpha*(u_t) is folded into the same PSUM group via an identity
matmul (u stored bf16), and alpha*b via a rank-1 matmul of a ones
vector — so the DVE only runs h' = 0.8*h + psum, then the elu feed
elu(h') = (min(exp(h'),1) - 1) + max(h',0) as two fused two-stage ops.
Each step's 72 matmuls are ordered in k<4 / k>=4 halves so the next
step always has runnable matmuls while this step's late elu chunks are
still in flight. One PSUM bank per m (a start=True clears its whole
bank, so concurrent groups must not share banks).

Structural masks: W_hi_eff is zero for h >= 512 (phase 1 computes only
m<4), W_oh_eff is zero for h < 512 (history/projection use only k>=4).
The scan is fully unrolled (no hw loop): all addresses static, no
back-edge barriers.
"""

import numpy as np

import concourse.bass as bass
import concourse.mybir as mybir
import concourse.tile as tile
from concourse.bass import ds, ts
from concourse.bass_utils import run_bass_kernel_spmd

AF = mybir.ActivationFunctionType
ALU = mybir.AluOpType
F32 = mybir.dt.float32
BF16 = mybir.dt.bfloat16


def _patch_tile_drain(max_drain_waits=1):
    """walrus CoreV3 codegen rejects the Tile end-of-kernel Drain when it
    carries several sem waits ("Too many sync wait commands"). Keep one wait
    on the drain and re-emit the rest as standalone SP wait_ge instructions;
    the all-engine barrier right after preserves semantics."""
    from concourse.vector_clock import ScopedClock

    if getattr(tile.TileContext, "_drain_patched", False):
        return

    def _drain_and_barrier(self, tick_clock, wait_clock):
        nc = self.nc
        drain_inst = nc.sync.drain()
        wait_clock.add_sem_waits(
            drain_inst.ins, ScopedClock({None: tick_clock.global_clock})
        )
        si = drain_inst.ins.sync_info
        waits = list(si.on_wait) if si is not None else []
        if len(waits) > max_drain_waits:
            si.on_wait = waits[:max_drain_waits]
            handles = {h.name: h for h in self.sems.allocated().values()}
            for w in waits[max_drain_waits:]:
                nc.sync.wait_ge(handles[w.ant_name], w.wait_value)

        nc.all_engine_barrier()
        assert self.sems is not None
        popped = nc._tile_sem_poison_stack.pop()
        assert popped is self._sem_poison
        nc.clear_and_free_semaphores(list(self.sems.allocated().values()))
        nc.all_engine_barrier()

    tile.TileContext._drain_and_barrier = _drain_and_barrier
    tile.TileContext._drain_patched = True


_patch_tile_drain()

WAIT_LIMIT = 1


def _split_excess_waits(nc, limit=WAIT_LIMIT):
    """walrus CoreV3 codegen accepts only one sem wait per instruction.
    Hoist the excess onto NoOp instructions inserted right before the
    offending instruction on the same engine."""
    def _nwaits(i):
        try:
            si = i.sync_info
        except Exception:
            return 0
        return len(si.on_wait) if si is not None else 0

    for f in nc.m.functions:
        for bb in f.blocks:
            il = bb.instructions
            if not any(_nwaits(i) > limit for i in il):
                continue
            new = []
            for inst in il:
                if _nwaits(inst) <= limit:
                    new.append(inst)
                    continue
                si = inst.sync_info
                waits = list(si.on_wait)
                extra = waits[:-limit]
                si.on_wait = waits[-limit:]
                for i0 in range(0, len(extra), limit):
                    nop = mybir.InstNoOp(
                        name=f"{inst.name}-wsplit{i0}",
                        engine=inst.engine,
                        bass_nofuse=True,
                        sync_info=mybir.SyncInfo(
                            on_wait=extra[i0:i0 + limit], on_update=[]
                        ),
                    )
                    new.append(nop)
                new.append(inst)
            il[:] = new


N_CORES = 8
B_FULL = 256
T_FULL = 512
H = 1024
D_IN = 66
O = 8
DT = 0.02
TAU = 0.1
ALPHA = DT / TAU          # 0.2
BL = B_FULL // N_CORES    # 32 batch per core
HK = H // 128             # 8 hidden chunks
MI = 4                    # input-driven m chunks (W_hi_eff zero for m>=4)
KO = 4                    # output-read k chunks  (W_oh_eff zero for k<4)
TB = 16                   # timesteps per phase-1/3 block (16*32 = 512 cols)


def build_program(T=T_FULL):
    """Builds the per-core Bass program (SPMD: same program on all cores)."""
    assert T % TB == 0
    nc = bass.Bass("TRN2", target_bir_lowering=False, debug=False,
                   num_devices=N_CORES)

    # ---- I/O ----
    xt = nc.dram_tensor("xt", [T, D_IN, BL], F32, kind="ExternalInput")
    whi = nc.dram_tensor("whi", [MI, D_IN, 128], F32, kind="ExternalInput")
    whh16 = nc.dram_tensor("whh16", [HK * HK, 128, 128], BF16,
                           kind="ExternalInput")
    woh16 = nc.dram_tensor("woh16", [KO, 128, O], BF16, kind="ExternalInput")
    cvec = nc.dram_tensor("cvec", [128, HK], F32, kind="ExternalInput")
    cv16 = nc.dram_tensor("cv16", [1, (HK - MI) * 128], BF16,
                          kind="ExternalInput")
    ident = nc.dram_tensor("ident", [128, 128], BF16, kind="ExternalInput")
    ones = nc.dram_tensor("ones", [1, BL], BF16, kind="ExternalInput")
    h0 = nc.dram_tensor("h0", [128, HK * BL], F32, kind="ExternalInput")
    eluh0 = nc.dram_tensor("eluh0", [128, HK * BL], BF16,
                           kind="ExternalInput")

    outs_t = nc.dram_tensor("outs_t", [O, T * BL], F32, kind="ExternalOutput")
    hlast = nc.dram_tensor("hlast", [128, HK * BL], F32, kind="ExternalOutput")

    with tile.TileContext(nc) as tc:
        with (
            tc.tile_pool(name="dram", bufs=1, space="DRAM") as dram,
            tc.tile_pool(name="const", bufs=1) as const,
            tc.tile_pool(name="state", bufs=1) as state,
            tc.tile_pool(name="ut", bufs=6) as ut_pool,
            tc.tile_pool(name="tmp", bufs=8) as tmp_pool,
            tc.tile_pool(name="h16", bufs=6) as h16_pool,
        ):
            u_dram = dram.tile([T, 128, MI * BL], BF16)
            hh2 = dram.tile([KO, T, 128, BL], BF16)  # h history, k-major

            # ---- constants into SBUF ----
            whi_sb = const.tile([D_IN, MI * 128], F32)
            nc.sync.dma_start(
                out=whi_sb[:].rearrange("d (m f) -> d m f", m=MI),
                in_=whi.rearrange("m d f -> d m f"),
            )
            w16_sb = const.tile([128, HK * HK * 128], BF16)
            nc.sync.dma_start(
                out=w16_sb[:].rearrange("p (n f) -> p n f", n=HK * HK),
                in_=whh16.rearrange("n p f -> p n f"),
            )
            woh_sb = const.tile([128, KO * O], BF16)
            nc.sync.dma_start(
                out=woh_sb[:].rearrange("p (k o) -> p k o", k=KO),
                in_=woh16.rearrange("k p o -> p k o"),
            )
            cv_sb = const.tile([128, HK], F32)
            nc.sync.dma_start(out=cv_sb[:], in_=cvec[:])
            cv16_sb = const.tile([1, (HK - MI) * 128], BF16)
            nc.sync.dma_start(out=cv16_sb[:], in_=cv16[:])
            id_sb = const.tile([128, 128], BF16)
            nc.sync.dma_start(out=id_sb[:], in_=ident[:])
            ones_sb = const.tile([1, BL], BF16)
            nc.sync.dma_start(out=ones_sb[:], in_=ones[:])

            h_a = state.tile([128, HK * BL], F32)
            h_b = state.tile([128, HK * BL], F32)
            eluh_a = state.tile([128, HK * BL], BF16)
            eluh_b = state.tile([128, HK * BL], BF16)
            nc.sync.dma_start(out=h_a[:], in_=h0[:])
            nc.sync.dma_start(out=eluh_a[:], in_=eluh0[:])

            # ---- phase 1: u''[t] = alpha*x_t@W_hi_eff^T + alpha*b, m<4 ----
            with (
                tc.tile_pool(name="xts", bufs=3) as xts_pool,
                tc.tile_pool(name="usb", bufs=4) as usb_pool,
                tc.tile_pool(name="psum_u", bufs=2, space="PSUM")
                as psum_u_pool,
            ):
                for tb in range(T // TB):
                    xts = xts_pool.tile([D_IN, TB * BL], F32)
                    nc.sync.dma_start(
                        out=xts[:].rearrange("d (t b) -> d t b", t=TB),
                        in_=xt[ts(tb, TB)].rearrange("t d b -> d t b"),
                    )
                    for m in range(MI):
                        pu = psum_u_pool.tile([128, TB * BL], F32)
                        nc.tensor.matmul(
                            pu[:], whi_sb[:, ts(m, 128)], xts[:],
                            start=True, stop=True,
                        )
                        usb = usb_pool.tile([128, TB * BL], BF16)
                        nc.scalar.activation(
                            usb[:], pu[:], AF.Identity,
                            bias=cv_sb[:, ds(m, 1)], scale=1.0,
                        )
                        nc.sync.dma_start(
                            out=u_dram[ts(tb, TB), :, ts(m, BL)]
                            .rearrange("t p b -> p t b"),
                            in_=usb[:].rearrange("p (t b) -> p t b", t=TB),
                        )

            # ---- phase 2: the recurrent scan (fully unrolled) ----
            def step(tj, h_in, eluh_in, h_out, eluh_out, psum_pool):
                ut = ut_pool.tile([128, MI * BL], BF16, name="ut")
                nc.sync.dma_start(out=ut[:], in_=u_dram[tj])
                pss = [psum_pool.tile([128, BL], F32, tag="ps", name="ps")
                       for _ in range(HK)]
                # half A: drive + k=0..3 for all m; half B: k=4..7
                for m in range(HK):
                    if m < MI:
                        nc.tensor.matmul(
                            pss[m][:], id_sb[:], ut[:, ts(m, BL)],
                            start=True, stop=False,
                        )
                    else:
                        nc.tensor.matmul(
                            pss[m][:], cv16_sb[:, ts(m - MI, 128)],
                            ones_sb[:],
                            start=True, stop=False,
                        )
                    for k in range(4):
                        nc.tensor.matmul(
                            pss[m][:], w16_sb[:, ts(m * HK + k, 128)],
                            eluh_in[:, ts(k, BL)],
                            start=False, stop=False,
                        )
                for m in range(HK):
                    for k in range(4, HK):
                        nc.tensor.matmul(
                            pss[m][:], w16_sb[:, ts(m * HK + k, 128)],
                            eluh_in[:, ts(k, BL)],
                            start=False, stop=(k == HK - 1),
                        )
                h16 = h16_pool.tile([128, KO * BL], BF16, name="h16")
                for P in range(4):
                    for i in range(2):
                        m = 2 * P + i
                        # h' = 0.8*h + (u'' + alpha*b + delta)
                        nc.vector.scalar_tensor_tensor(
                            h_out[:, ts(m, BL)], h_in[:, ts(m, BL)],
                            float(1.0 - ALPHA), pss[m][:],
                            ALU.mult, ALU.add,
                        )
                    cp = ds(P * 2 * BL, 2 * BL)
                    # rm1 = max(h',0) - 1
                    rm1 = tmp_pool.tile([128, 2 * BL], F32, tag="rm1",
                                        name="rm1")
                    nc.vector.tensor_scalar(rm1[:], h_out[:, cp], 0.0, -1.0,
                                            ALU.max, ALU.add)
                    e = tmp_pool.tile([128, 2 * BL], F32, tag="e", name="e")
                    nc.scalar.activation(e[:], h_out[:, cp], AF.Exp)
                    # elu(h') = (min(exp(h'),1) - 1) + max(h',0)
                    nc.vector.scalar_tensor_tensor(
                        eluh_out[:, cp], e[:], 1.0, rm1[:],
                        ALU.min, ALU.add,
                    )
                    if P == 3:
                        # history for the output projection: chunks k>=4
                        nc.scalar.activation(
                            h16[:], h_out[:, ds((HK - KO) * BL, KO * BL)],
                            AF.Copy, bias=0.0, scale=1.0,
                        )
                nc.sync.dma_start(
                    out=hh2[:, tj].rearrange("k p b -> p k b"),
                    in_=h16[:].rearrange("p (k b) -> p k b", k=KO),
                )

            with tc.tile_pool(name="psum", bufs=8, space="PSUM") as psum_pool:
                for t in range(T):
                    if t % 2 == 0:
                        step(t, h_a, eluh_a, h_b, eluh_b, psum_pool)
                    else:
                        step(t, h_b, eluh_b, h_a, eluh_a, psum_pool)

            # after T steps (T even) the live state is back in h_a
            nc.sync.dma_start(out=hlast[:], in_=h_a[:])

            # ---- phase 3: outs_t = W_oh_eff @ h_hist^T (k>=4 only) ----
            with (
                tc.tile_pool(name="hh", bufs=4) as hh_pool,
                tc.tile_pool(name="ob", bufs=4) as ob_pool,
                tc.tile_pool(name="psum_o", bufs=2, space="PSUM")
                as psum_o_pool,
            ):
                for nb in range(T // TB):
                    po = psum_o_pool.tile([O, TB * BL], F32)
                    for k in range(KO):
                        hhs = hh_pool.tile([128, TB * BL], BF16, name="hhs")
                        nc.sync.dma_start(
                            out=hhs[:].rearrange("p (t b) -> p t b", t=TB),
                            in_=hh2[k, ts(nb, TB)].rearrange("t p b -> p t b"),
                        )
                        nc.tensor.matmul(
                            po[:], woh_sb[:, ts(k, O)], hhs[:],
                            start=(k == 0), stop=(k == KO - 1),
                        )
                    ob = ob_pool.tile([O, TB * BL], F32)
                    nc.vector.tensor_copy(ob[:], po[:])
                    nc.sync.dma_start(out=outs_t[:, ts(nb, TB * BL)],
                                      in_=ob[:])

    _split_excess_waits(nc)
    return nc


def _prep_core_inputs(x_shard, W_hi, W_hh, b, W_oh, hidden_init):
    """Host-side packing of one core's inputs (cheap O(H^2) work)."""
    import ml_dtypes

    half = H // 2
    row = np.arange(H)
    W_hi_eff = np.where(row[:, None] < half, W_hi, 0.0).astype(np.float32)
    W_oh_eff = np.where(row[None, :] >= half, W_oh, 0.0).astype(np.float32)

    xtp = np.ascontiguousarray(x_shard.transpose(1, 2, 0))  # [T, D, BL]

    aWhi = (ALPHA * W_hi_eff).astype(np.float32)
    whi = np.ascontiguousarray(
        aWhi.reshape(HK, 128, D_IN)[:MI].transpose(0, 2, 1)
    )  # [MI, D, 128]

    aWt = (ALPHA * W_hh).astype(np.float32).T  # [h_in, h_out]
    whh16 = np.zeros((HK * HK, 128, 128), np.float32)
    for m in range(HK):
        for k in range(HK):
            whh16[m * HK + k] = aWt[k * 128:(k + 1) * 128,
                                    m * 128:(m + 1) * 128]
    whh16 = whh16.astype(ml_dtypes.bfloat16)

    woh16 = np.ascontiguousarray(
        W_oh_eff.T.reshape(HK, 128, O)[KO:]
    ).astype(ml_dtypes.bfloat16)  # [KO, 128, O] for k>=4

    # cvec[p, m] = (alpha*b)[128m + p]
    cv = (ALPHA * b).astype(np.float32)
    cvec = np.ascontiguousarray(cv.reshape(HK, 128).T)
    cv16 = np.ascontiguousarray(
        cv.reshape(HK, 128)[MI:].reshape(1, (HK - MI) * 128)
    ).astype(ml_dtypes.bfloat16)

    ident = np.eye(128, dtype=np.float32).astype(ml_dtypes.bfloat16)
    ones = np.ones((1, BL), np.float32).astype(ml_dtypes.bfloat16)

    # P1 packing of the initial state: [p, 32k + b] = h[b, 128k + p]
    h0_full = np.broadcast_to(hidden_init, (BL, H)).astype(np.float32)
    h0 = np.ascontiguousarray(
        h0_full.reshape(BL, HK, 128).transpose(2, 1, 0).reshape(128, HK * BL)
    )
    feed0 = np.maximum(h0, 0.0) + np.expm1(np.minimum(h0, 0.0))
    eluh0 = feed0.astype(ml_dtypes.bfloat16)

    return {
        "xt": xtp, "whi": whi, "whh16": whh16, "woh16": woh16,
        "cvec": cvec, "cv16": cv16, "ident": ident, "ones": ones,
        "h0": h0, "eluh0": eluh0,
    }


_PROGRAM_CACHE = {}


def run(x, W_hi, W_hh, b, W_oh, hidden_init, T=None, trace=False):
    x = np.asarray(x, np.float32)
    B, T_in, _ = x.shape
    if T is None:
        T = T_in
    assert B == B_FULL
    if T not in _PROGRAM_CACHE:
        _PROGRAM_CACHE[T] = build_program(T)
    nc = _PROGRAM_CACHE[T]

    in_maps = []
    for c in range(N_CORES):
        shard = x[c * BL:(c + 1) * BL, :T]
        in_maps.append(_prep_core_inputs(
            shard, np.asarray(W_hi, np.float32), np.asarray(W_hh, np.float32),
            np.asarray(b, np.float32), np.asarray(W_oh, np.float32),
            np.asarray(hidden_init, np.float32),
        ))

    res = run_bass_kernel_spmd(nc, in_maps, core_ids=list(range(N_CORES)),
                               trace=trace)

    outs = np.empty((B_FULL, T, O), np.float32)
    h_last = np.empty((B_FULL, H), np.float32)
    for c in range(N_CORES):
        ot = res.results[c]["outs_t"]          # [O, T*BL]
        outs[c * BL:(c + 1) * BL] = (
            ot.reshape(O, T, BL).transpose(2, 1, 0)
        )
        hl = res.results[c]["hlast"]           # [128, HK*BL]
        h_last[c * BL:(c + 1) * BL] = (
            hl.reshape(128, HK, BL).transpose(2, 1, 0).reshape(BL, H)
        )
    return (outs, h_last), res


def kernel(x, W_hi, W_hh, b, W_oh, hidden_init):
    (outs, h_last), _ = run(x, W_hi, W_hh, b, W_oh, hidden_init)
    return outs, h_last


# revision 22
# speedup vs baseline: 1.9113x; 1.1378x over previous
"""Trainium2 Bass kernel for the BiologicalRNN problem.

Reference semantics:
    u_seq = einsum('btd,hd->tbh', x, W_hi_eff)        # input drive
    h_{t+1} = h_t + alpha*(-h_t + u_t + elu(h_t) @ W_hh^T + b)
    outs[t] = h_{t+1} @ W_oh_eff^T
    returns (outs.transpose(1,0,2), h_last)

Strategy: data-parallel over batch B=256 across 8 NeuronCores (32 each).
Per core everything lives in a transposed "P1" layout: a [128, 256] tile
where element [p, 32*k + b] = v[b, 128*k + p] for a [32, 1024] matrix.

The recurrent matmul runs weights-stationary in bf16: for each output
chunk m, psum[m] [128,32] accumulates 8 contraction chunks k with
lhsT = (alpha*W_hh)^T tile (k,m) and rhs = elu(h)^T chunk k. The drive
u''_t = alpha*(u_t) is folded into the same PSUM group via an identity
matmul (u stored bf16), and alpha*b via a rank-1 matmul of a ones
vector — so the DVE only runs h' = 0.8*h + psum, then the elu feed
elu(h') = (min(exp(h'),1) - 1) + max(h',0) as two fused two-stage ops.
Each step's 72 matmuls are ordered in k<4 / k>=4 halves so the next
step always has runnable matmuls while this step's late elu chunks are
still in flight. One PSUM bank per m (a start=True clears its whole
bank, so concurrent groups must not share banks).

Structural masks: W_hi_eff is zero for h >= 512 (phase 1 computes only
m<4), W_oh_eff is zero for h < 512 (history/projection use only k>=4).
The scan is fully unrolled (no hw loop): all addresses static, no
back-edge barriers.
"""

import numpy as np

import concourse.bass as bass
import concourse.mybir as mybir
import concourse.tile as tile
from concourse.bass import ds, ts
from concourse.bass_utils import run_bass_kernel_spmd

AF = mybir.ActivationFunctionType
ALU = mybir.AluOpType
F32 = mybir.dt.float32
BF16 = mybir.dt.bfloat16
F32R = mybir.dt.float32r


def _patch_tile_drain(max_drain_waits=1):
    """walrus CoreV3 codegen rejects the Tile end-of-kernel Drain when it
    carries several sem waits ("Too many sync wait commands"). Keep one wait
    on the drain and re-emit the rest as standalone SP wait_ge instructions;
    the all-engine barrier right after preserves semantics."""
    from concourse.vector_clock import ScopedClock

    if getattr(tile.TileContext, "_drain_patched", False):
        return

    def _drain_and_barrier(self, tick_clock, wait_clock):
        nc = self.nc
        drain_inst = nc.sync.drain()
        wait_clock.add_sem_waits(
            drain_inst.ins, ScopedClock({None: tick_clock.global_clock})
        )
        si = drain_inst.ins.sync_info
        waits = list(si.on_wait) if si is not None else []
        if len(waits) > max_drain_waits:
            si.on_wait = waits[:max_drain_waits]
            handles = {h.name: h for h in self.sems.allocated().values()}
            for w in waits[max_drain_waits:]:
                nc.sync.wait_ge(handles[w.ant_name], w.wait_value)

        nc.all_engine_barrier()
        assert self.sems is not None
        popped = nc._tile_sem_poison_stack.pop()
        assert popped is self._sem_poison
        nc.clear_and_free_semaphores(list(self.sems.allocated().values()))
        nc.all_engine_barrier()

    tile.TileContext._drain_and_barrier = _drain_and_barrier
    tile.TileContext._drain_patched = True


_patch_tile_drain()

WAIT_LIMIT = 1


def _split_excess_waits(nc, limit=WAIT_LIMIT):
    """walrus CoreV3 codegen accepts only one sem wait per instruction.
    Hoist the excess onto NoOp instructions inserted right before the
    offending instruction on the same engine."""
    def _nwaits(i):
        try:
            si = i.sync_info
        except Exception:
            return 0
        return len(si.on_wait) if si is not None else 0

    for f in nc.m.functions:
        for bb in f.blocks:
            il = bb.instructions
            if not any(_nwaits(i) > limit for i in il):
                continue
            new = []
            for inst in il:
                if _nwaits(inst) <= limit:
                    new.append(inst)
                    continue
                si = inst.sync_info
                waits = list(si.on_wait)
                extra = waits[:-limit]
                si.on_wait = waits[-limit:]
                for i0 in range(0, len(extra), limit):
                    nop = mybir.InstNoOp(
                        name=f"{inst.name}-wsplit{i0}",
                        engine=inst.engine,
                        bass_nofuse=True,
                        sync_info=mybir.SyncInfo(
                            on_wait=extra[i0:i0 + limit], on_update=[]
                        ),
                    )
                    new.append(nop)
                new.append(inst)
            il[:] = new


N_CORES = 8
B_FULL = 256
T_FULL = 512
H = 1024
D_IN = 66
O = 8
DT = 0.02
TAU = 0.1
ALPHA = DT / TAU          # 0.2
BL = B_FULL // N_CORES    # 32 batch per core
HK = H // 128             # 8 hidden chunks
MI = 4                    # input-driven m chunks (W_hi_eff zero for m>=4)
KO = 4                    # output-read k chunks  (W_oh_eff zero for k<4)
TB = 16                   # timesteps per phase-1/3 block (16*32 = 512 cols)


def build_program(T=T_FULL):
    """Builds the per-core Bass program (SPMD: same program on all cores)."""
    assert T % TB == 0
    nc = bass.Bass("TRN2", target_bir_lowering=False, debug=False,
                   num_devices=N_CORES)

    # ---- I/O ----
    xt = nc.dram_tensor("xt", [T, D_IN, BL], F32R, kind="ExternalInput")
    whi = nc.dram_tensor("whi", [MI, D_IN, 128], F32R,
                         kind="ExternalInput")
    whh16 = nc.dram_tensor("whh16", [HK * HK, 128, 128], BF16,
                           kind="ExternalInput")
    woh16 = nc.dram_tensor("woh16", [KO, 128, O], BF16, kind="ExternalInput")
    cvec = nc.dram_tensor("cvec", [128, HK], F32, kind="ExternalInput")
    cv16 = nc.dram_tensor("cv16", [1, (HK - MI) * 128], BF16,
                          kind="ExternalInput")
    ident = nc.dram_tensor("ident", [128, 128], BF16, kind="ExternalInput")
    ones = nc.dram_tensor("ones", [1, BL], BF16, kind="ExternalInput")
    h0 = nc.dram_tensor("h0", [128, HK * BL], F32, kind="ExternalInput")
    eluh0 = nc.dram_tensor("eluh0", [128, HK * BL], BF16,
                           kind="ExternalInput")

    outs_t = nc.dram_tensor("outs_t", [O, T * BL], F32, kind="ExternalOutput")
    hlast = nc.dram_tensor("hlast", [128, HK * BL], F32, kind="ExternalOutput")

    with tile.TileContext(nc) as tc:
        with (
            tc.tile_pool(name="dram", bufs=1, space="DRAM") as dram,
            tc.tile_pool(name="const", bufs=1) as const,
            tc.tile_pool(name="state", bufs=1) as state,
            tc.tile_pool(name="ut", bufs=6) as ut_pool,
            tc.tile_pool(name="tmp", bufs=8) as tmp_pool,
            tc.tile_pool(name="h16", bufs=6) as h16_pool,
        ):
            u_dram = dram.tile([T, 128, MI * BL], BF16)
            hh2 = dram.tile([KO, T, 128, BL], BF16)  # h history, k-major

            # ---- constants into SBUF ----
            whi_sb = const.tile([D_IN, MI * 128], F32R)
            nc.sync.dma_start(
                out=whi_sb[:].rearrange("d (m f) -> d m f", m=MI),
                in_=whi.rearrange("m d f -> d m f"),
            )
            w16_sb = const.tile([128, HK * HK * 128], BF16)
            nc.sync.dma_start(
                out=w16_sb[:].rearrange("p (n f) -> p n f", n=HK * HK),
                in_=whh16.rearrange("n p f -> p n f"),
            )
            woh_sb = const.tile([128, KO * O], BF16)
            nc.sync.dma_start(
                out=woh_sb[:].rearrange("p (k o) -> p k o", k=KO),
                in_=woh16.rearrange("k p o -> p k o"),
            )
            cv_sb = const.tile([128, HK], F32)
            nc.sync.dma_start(out=cv_sb[:], in_=cvec[:])
            cv16_sb = const.tile([1, (HK - MI) * 128], BF16)
            nc.sync.dma_start(out=cv16_sb[:], in_=cv16[:])
            id_sb = const.tile([128, 128], BF16)
            nc.sync.dma_start(out=id_sb[:], in_=ident[:])
            ones_sb = const.tile([1, BL], BF16)
            nc.sync.dma_start(out=ones_sb[:], in_=ones[:])

            h_a = state.tile([128, HK * BL], F32)
            h_b = state.tile([128, HK * BL], F32)
            eluh_a = state.tile([128, HK * BL], BF16)
            eluh_b = state.tile([128, HK * BL], BF16)
            nc.sync.dma_start(out=h_a[:], in_=h0[:])
            nc.sync.dma_start(out=eluh_a[:], in_=eluh0[:])

            # ---- phase 1: u''[t] = alpha*x_t@W_hi_eff^T + alpha*b, m<4 ----
            with (
                tc.tile_pool(name="xts", bufs=3) as xts_pool,
                tc.tile_pool(name="usb", bufs=4) as usb_pool,
                tc.tile_pool(name="psum_u", bufs=2, space="PSUM")
                as psum_u_pool,
            ):
                for tb in range(T // TB):
                    xts = xts_pool.tile([D_IN, TB * BL], F32R)
                    nc.sync.dma_start(
                        out=xts[:].rearrange("d (t b) -> d t b", t=TB),
                        in_=xt[ts(tb, TB)].rearrange("t d b -> d t b"),
                    )
                    for m in range(MI):
                        pu = psum_u_pool.tile([128, TB * BL], F32)
                        nc.tensor.matmul(
                            pu[:], whi_sb[:, ts(m, 128)], xts[:],
                            start=True, stop=True,
                        )
                        usb = usb_pool.tile([128, TB * BL], BF16)
                        nc.scalar.activation(
                            usb[:], pu[:], AF.Identity,
                            bias=cv_sb[:, ds(m, 1)], scale=1.0,
                        )
                        nc.sync.dma_start(
                            out=u_dram[ts(tb, TB), :, ts(m, BL)]
                            .rearrange("t p b -> p t b"),
                            in_=usb[:].rearrange("p (t b) -> p t b", t=TB),
                        )

            # ---- phase 2: the recurrent scan (fully unrolled) ----
            def step(tj, h_in, eluh_in, h_out, eluh_out, psum_pool):
                ut = ut_pool.tile([128, MI * BL], BF16, name="ut")
                nc.sync.dma_start(out=ut[:], in_=u_dram[tj])
                pss = [psum_pool.tile([128, BL], F32, tag="ps", name="ps")
                       for _ in range(HK)]
                # half A: drive + k=0..3 for all m; half B: k=4..7
                for m in range(HK):
                    if m < MI:
                        nc.tensor.matmul(
                            pss[m][:], id_sb[:], ut[:, ts(m, BL)],
                            start=True, stop=False,
                        )
                    else:
                        nc.tensor.matmul(
                            pss[m][:], cv16_sb[:, ts(m - MI, 128)],
                            ones_sb[:],
                            start=True, stop=False,
                        )
                    for k in range(4):
                        nc.tensor.matmul(
                            pss[m][:], w16_sb[:, ts(m * HK + k, 128)],
                            eluh_in[:, ts(k, BL)],
                            start=False, stop=False,
                        )
                for m in range(HK):
                    for k in range(4, HK):
                        nc.tensor.matmul(
                            pss[m][:], w16_sb[:, ts(m * HK + k, 128)],
                            eluh_in[:, ts(k, BL)],
                            start=False, stop=(k == HK - 1),
                        )
                h16 = h16_pool.tile([128, KO * BL], BF16, name="h16")
                for c in range(2):
                    for i in range(4):
                        m = 4 * c + i
                        # h' = 0.8*h + (u'' + alpha*b + delta)
                        nc.vector.scalar_tensor_tensor(
                            h_out[:, ts(m, BL)], h_in[:, ts(m, BL)],
                            float(1.0 - ALPHA), pss[m][:],
                            ALU.mult, ALU.add,
                        )
                    cp = ds(c * 4 * BL, 4 * BL)
                    # rm1 = max(h',0) - 1
                    rm1 = tmp_pool.tile([128, 4 * BL], F32, tag="rm1",
                                        name="rm1")
                    nc.vector.tensor_scalar(rm1[:], h_out[:, cp], 0.0, -1.0,
                                            ALU.max, ALU.add)
                    e = tmp_pool.tile([128, 4 * BL], F32, tag="e", name="e")
                    nc.scalar.activation(e[:], h_out[:, cp], AF.Exp)
                    # elu(h') = (min(exp(h'),1) - 1) + max(h',0)
                    nc.vector.scalar_tensor_tensor(
                        eluh_out[:, cp], e[:], 1.0, rm1[:],
                        ALU.min, ALU.add,
                    )
                    if c == 1:
                        # history for the output projection: chunks k>=4
                        nc.scalar.activation(
                            h16[:], h_out[:, ds((HK - KO) * BL, KO * BL)],
                            AF.Copy, bias=0.0, scale=1.0,
                        )
                nc.sync.dma_start(
                    out=hh2[:, tj].rearrange("k p b -> p k b"),
                    in_=h16[:].rearrange("p (k b) -> p k b", k=KO),
                )

            with tc.tile_pool(name="psum", bufs=8, space="PSUM") as psum_pool:
                for t in range(T):
                    if t % 2 == 0:
                        step(t, h_a, eluh_a, h_b, eluh_b, psum_pool)
                    else:
                        step(t, h_b, eluh_b, h_a, eluh_a, psum_pool)

            # after T steps (T even) the live state is back in h_a
            nc.sync.dma_start(out=hlast[:], in_=h_a[:])

            # ---- phase 3: outs_t = W_oh_eff @ h_hist^T (k>=4 only) ----
            with (
                tc.tile_pool(name="hh", bufs=4) as hh_pool,
                tc.tile_pool(name="ob", bufs=4) as ob_pool,
                tc.tile_pool(name="psum_o", bufs=2, space="PSUM")
                as psum_o_pool,
            ):
                for nb in range(T // TB):
                    po = psum_o_pool.tile([O, TB * BL], F32)
                    for k in range(KO):
                        hhs = hh_pool.tile([128, TB * BL], BF16, name="hhs")
                        nc.sync.dma_start(
                            out=hhs[:].rearrange("p (t b) -> p t b", t=TB),
                            in_=hh2[k, ts(nb, TB)].rearrange("t p b -> p t b"),
                        )
                        nc.tensor.matmul(
                            po[:], woh_sb[:, ts(k, O)], hhs[:],
                            start=(k == 0), stop=(k == KO - 1),
                        )
                    ob = ob_pool.tile([O, TB * BL], F32)
                    nc.vector.tensor_copy(ob[:], po[:])
                    nc.sync.dma_start(out=outs_t[:, ts(nb, TB * BL)],
                                      in_=ob[:])

    _split_excess_waits(nc)
    return nc


def _prep_core_inputs(x_shard, W_hi, W_hh, b, W_oh, hidden_init):
    """Host-side packing of one core's inputs (cheap O(H^2) work)."""
    import ml_dtypes

    half = H // 2
    row = np.arange(H)
    W_hi_eff = np.where(row[:, None] < half, W_hi, 0.0).astype(np.float32)
    W_oh_eff = np.where(row[None, :] >= half, W_oh, 0.0).astype(np.float32)

    xtp = np.ascontiguousarray(x_shard.transpose(1, 2, 0))  # [T, D, BL]

    aWhi = (ALPHA * W_hi_eff).astype(np.float32)
    whi = np.ascontiguousarray(
        aWhi.reshape(HK, 128, D_IN)[:MI].transpose(0, 2, 1)
    )  # [MI, D, 128]

    aWt = (ALPHA * W_hh).astype(np.float32).T  # [h_in, h_out]
    whh16 = np.zeros((HK * HK, 128, 128), np.float32)
    for m in range(HK):
        for k in range(HK):
            whh16[m * HK + k] = aWt[k * 128:(k + 1) * 128,
                                    m * 128:(m + 1) * 128]
    whh16 = whh16.astype(ml_dtypes.bfloat16)

    woh16 = np.ascontiguousarray(
        W_oh_eff.T.reshape(HK, 128, O)[KO:]
    ).astype(ml_dtypes.bfloat16)  # [KO, 128, O] for k>=4

    # cvec[p, m] = (alpha*b)[128m + p]
    cv = (ALPHA * b).astype(np.float32)
    cvec = np.ascontiguousarray(cv.reshape(HK, 128).T)
    cv16 = np.ascontiguousarray(
        cv.reshape(HK, 128)[MI:].reshape(1, (HK - MI) * 128)
    ).astype(ml_dtypes.bfloat16)

    ident = np.eye(128, dtype=np.float32).astype(ml_dtypes.bfloat16)
    ones = np.ones((1, BL), np.float32).astype(ml_dtypes.bfloat16)

    # P1 packing of the initial state: [p, 32k + b] = h[b, 128k + p]
    h0_full = np.broadcast_to(hidden_init, (BL, H)).astype(np.float32)
    h0 = np.ascontiguousarray(
        h0_full.reshape(BL, HK, 128).transpose(2, 1, 0).reshape(128, HK * BL)
    )
    feed0 = np.maximum(h0, 0.0) + np.expm1(np.minimum(h0, 0.0))
    eluh0 = feed0.astype(ml_dtypes.bfloat16)

    return {
        "xt": xtp, "whi": whi, "whh16": whh16, "woh16": woh16,
        "cvec": cvec, "cv16": cv16, "ident": ident, "ones": ones,
        "h0": h0, "eluh0": eluh0,
    }


_PROGRAM_CACHE = {}


def run(x, W_hi, W_hh, b, W_oh, hidden_init, T=None, trace=False):
    x = np.asarray(x, np.float32)
    B, T_in, _ = x.shape
    if T is None:
        T = T_in
    assert B == B_FULL
    if T not in _PROGRAM_CACHE:
        _PROGRAM_CACHE[T] = build_program(T)
    nc = _PROGRAM_CACHE[T]

    in_maps = []
    for c in range(N_CORES):
        shard = x[c * BL:(c + 1) * BL, :T]
        in_maps.append(_prep_core_inputs(
            shard, np.asarray(W_hi, np.float32), np.asarray(W_hh, np.float32),
            np.asarray(b, np.float32), np.asarray(W_oh, np.float32),
            np.asarray(hidden_init, np.float32),
        ))

    res = run_bass_kernel_spmd(nc, in_maps, core_ids=list(range(N_CORES)),
                               trace=trace)

    outs = np.empty((B_FULL, T, O), np.float32)
    h_last = np.empty((B_FULL, H), np.float32)
    for c in range(N_CORES):
        ot = res.results[c]["outs_t"]          # [O, T*BL]
        outs[c * BL:(c + 1) * BL] = (
            ot.reshape(O, T, BL).transpose(2, 1, 0)
        )
        hl = res.results[c]["hlast"]           # [128, HK*BL]
        h_last[c * BL:(c + 1) * BL] = (
            hl.reshape(128, HK, BL).transpose(2, 1, 0).reshape(BL, H)
        )
    return (outs, h_last), res


def kernel(x, W_hi, W_hh, b, W_oh, hidden_init):
    (outs, h_last), _ = run(x, W_hi, W_hh, b, W_oh, hidden_init)
    return outs, h_last
